# revision 1
# baseline (speedup 1.0000x reference)
"""Trainium2 Bass kernel for ChannelSelection (top-k channel masking).

Reference computation (per vehicle n of N=4):
  s = 0.5*grad_mag(x) + 0.5*|x|            # grad_mag = |x[w+1]-x[w-1]| + |x[h+1]-x[h-1]|
  sp[c, patch] = mean of s over 32x32 patch
  keep top-128 (of 256) channels per patch (rank by sp desc, stable)
  out = x * mask broadcast over patch

Sharding: 8 cores = N(4) x H-halves(2). Each core gets one vehicle's
channel-complete slab of 128 rows (+1 halo row each side, zero padded
by the host) and computes its patches' top-k independently (patches
never straddle the H split since 128 % 32 == 0).

Structure (per core): 16 units = 4 row-strips x 2 column-halves x 2
channel-groups, each a [128, 34, 130] SBUF tile (1px halo baked in),
ring of 8. Units stream load -> grad/abs-accumulate; after both channel
groups of a (strip, half) land, that half's 4 patches are ranked and
masked in place -> store. Input DMAs issue on the SP HWDGE queue,
output DMAs on the ACT HWDGE queue, deferred by one half so the ACT
sequencer never stalls waiting for DVE maskmuls.

Engine balance (per core, cost-model ns/elem; only ISA-legal ops --
tensor_scalar(abs_max) and Pool accumulations are rejected by the
NEFF compiler):
  DVE  : ex = x[w+1]-x[w-1] TT; |x| patch sums as one 4-patch
         tensor_reduce(abs) per g1 unit + one 2-patch ey reduce;
         rank counts via is_gt+accum vs the PE broadcast; maskmul via
         2x tensor_scalar mult (in place)
  ACT  : per-patch Abs+accum of ex/ey (and g0 |x|)
  GPSIMD: ey = x[h+1]-x[h-1] TT
  PE   : transpose sp + one-hot broadcast matmul for the rank compare
  DMA floor ~194us is the target bottleneck (in 36.2MB + out 33.5MB
  per core at ~360 GB/s).
"""

import dataclasses
import os
import sys

import numpy as np

_TRN_REPO = "/opt/trn_rl_repo"
if _TRN_REPO not in sys.path:
    sys.path.insert(0, _TRN_REPO)

# Full-problem constants (hardcoded per contest rules)
N_VEH = 4
C = 256
H = 256
W = 256
P = 32          # patch size
N_CORES = 8
HS = 128        # rows per core (H/2)

_cache = {}


def build_program(ns=4, npc=8, xbufs=7, gq=2, gq1=None, tq0=0, tq1=0, dq0=1,
                  exd0=0, exd1=0, eyd0=0, eyd1=1, xd0=0, xd1=1,
                  msplit=4, sign_dve=True, stage=1, stbufs=2, s0_dve=0,
                  out_defer=1, exbufs=2, eybufs=2, eyorder=0, rank_hi=0, dq_last=0, ey_first=0, s3_act=0, osplit=0, msplit_last=None, msl2=0, tq_s0=None, tq_s0_span=0, reps=1):
    """Build the SPMD Bass program for one core.

    ns:      number of 32-row strips (ns*32 = rows per core)
    npc:     number of patch columns (npc*32 = W)
    xbufs:   x-tile ring depth (units in flight)
    gq:      ey-subtract quarters per unit on GPSIMD, rest DVE (0..2)
    tq0/tq1: ex-subtract quarters per unit on GPSIMD by group (0..2)
    dq0:     deferred GPSIMD ex-quarters for the g0 unit of each half,
             emitted after the g1 unit so Pool lag misses the barrier
    exd0/1:  ex abs-acc quarters per unit on DVE (2-patch reduce) by
             group, rest ACT per-patch Abs+accum (0..2)
    eyd0/1:  same for ey abs-acc
    xd0/1:   |x| abs-acc on DVE as one 4-patch reduce (1) or ACT (0)
    msplit:  maskmul patches per unit on DVE (2x tensor_scalar), rest
             ACT Copy+scale (0..4)
    sign_dve: rank count via DVE is_gt+accum instead of ACT Sign+accum
    stage:   maskmul writes a separate staging tile (frees x at maskmul
             time instead of out-DMA completion); 0 = in-place
    stbufs:  staging ring depth
    s0_dve:  strip 0 runs its ey subs on DVE (fill-phase fast track)
    reps:    repeat the whole pipeline (timing harness only)
    """
    from contextlib import ExitStack

    import concourse.bass as bass
    import concourse.tile as tile
    from concourse import bacc, masks, mybir

    f32 = mybir.dt.float32
    Alu = mybir.AluOpType
    Act = mybir.ActivationFunctionType

    rows = ns * P
    w = npc * P
    wp = w + 2
    hw_ = w // 2          # out cols per half (128)
    nph = npc // 2        # patches per half (4)

    nc = bacc.Bacc("TRN2", target_bir_lowering=False, debug=False)
    x_ap = nc.dram_tensor("x", [C, rows + 2, wp], f32, kind="ExternalInput").ap()
    oh_ap = nc.dram_tensor(
        "onehot", [nph, nph * 128], f32, kind="ExternalInput"
    ).ap()
    o_ap = nc.dram_tensor("out", [C, rows, w], f32, kind="ExternalOutput").ap()

    with tile.TileContext(nc) as tc, ExitStack() as ctx:
        const_pool = ctx.enter_context(tc.tile_pool(name="const", bufs=1))
        x_pool = ctx.enter_context(tc.tile_pool(name="xs", bufs=xbufs))
        ex_pool = ctx.enter_context(tc.tile_pool(name="ex", bufs=exbufs))
        ey_pool = ctx.enter_context(tc.tile_pool(name="ey", bufs=eybufs))
        if stage:
            # out_defer keeps a staging tile alive across the next half's
            # maskmuls -- need at least 4 slots to avoid aliasing
            if out_defer:
                stbufs = max(stbufs, 4)
            st_pool = ctx.enter_context(tc.tile_pool(name="st", bufs=stbufs))
        acc_pool = ctx.enter_context(tc.tile_pool(name="acc", bufs=16))
        spt_pool = ctx.enter_context(tc.tile_pool(name="spt", bufs=3))
        scr_pool = ctx.enter_context(tc.tile_pool(name="scr", bufs=1))
        ps_b = ctx.enter_context(tc.tile_pool(name="ps_b", bufs=5, space="PSUM"))
        ps_sgn = ctx.enter_context(tc.tile_pool(name="ps_sgn", bufs=1, space="PSUM"))
        ps_t = ctx.enter_context(tc.tile_pool(name="ps_t", bufs=2, space="PSUM"))

        # constants
        ident = const_pool.tile([128, 128], f32)
        masks.make_identity(nc, ident[:])
        # onehot[p, 128p:128(p+1)] = 1: stationary for the patch-row broadcast
        onehot = const_pool.tile([nph, nph * 128], f32)
        nc.scalar.dma_start(onehot[:], oh_ap[:])

        # engine-private scratch (mandatory elementwise outputs, discarded)
        dve_dump = scr_pool.tile([128, P, P], f32, tag="dve_dump")
        act_dump = scr_pool.tile([128, P, P], f32, tag="act_dump")
        _dd = dve_dump[:, :, :]
        dve_dump2 = dataclasses.replace(_dd, ap=[_dd.ap[0], [1, 256]])

        def absacc_act(src, accum):
            """accum[:,0] = per-partition sum |src| on ACT."""
            nc.scalar.activation(act_dump[:], src, Act.Abs, accum_out=accum)

        def absacc_mp(src, np_, row_stride, accum):
            """accum[:, 0:np_] = per-patch sum |src| via one DVE
            tensor_reduce over a [128, np_, P, P] view of src (src is a
            [128, P, np_*P] region with the given row stride)."""
            v = dataclasses.replace(
                src, ap=[src.ap[0], [P, np_], [row_stride, P], [1, P]]
            )
            nc.vector.tensor_reduce(
                accum, v, axis=mybir.AxisListType.XY, op=Alu.add,
                apply_absolute_value=True,
            )

        for rep in range(reps):

            def emit_unit(s, h, g):
                """Load + grad + abs-accum for unit (strip s, half h,
                group g). Returns (x tile, sp accum [128, nph])."""
                r0 = s * P
                t = x_pool.tile([128, P + 2, hw_ + 2], f32, tag="x",
                                name=f"x{g}{h}")
                nc.sync.dma_start(
                    t[:],
                    x_ap[g * 128:(g + 1) * 128, r0:r0 + P + 2,
                         h * hw_:h * hw_ + hw_ + 2],
                )
                exs = acc_pool.tile([128, nph], f32, tag="exs",
                                    name=f"exs{g}{h}")
                eys = acc_pool.tile([128, nph], f32, tag="eys",
                                    name=f"eys{g}{h}")
                xs_ = acc_pool.tile([128, nph], f32, tag="xs",
                                    name=f"xs{g}{h}")
                tq = (tq0, tq1)[g]
                if tq_s0 is not None and s <= tq_s0_span:
                    # fill: Pool starts cold; keep strip 0's ex on DVE
                    tq = tq_s0
                exd = (exd0, exd1)[g]
                eyd = (eyd0, eyd1)[g]
                xd = (xd0, xd1)[g]
                if s3_act and s == ns - 1:
                    # drain: DVE's queue tail is the longest; push the
                    # last strip's reduce-accs to ACT instead
                    eyd = 0
                    xd = 0
                deferred = []
                ey_accs = []
                eyts = {}
                if ey_first:
                    # emit both ey subtracts first so a Pool-assigned ex
                    # quarter doesn't delay the ey chain
                    for q in range(2):
                        c0 = 64 * q
                        eyt = ey_pool.tile([128, P, 64], f32, tag="eyt")
                        gqg = gq if (g == 0 or gq1 is None) else gq1
                        on_pool = q < gqg and not (s == 0 and s0_dve)
                        eng = nc.gpsimd if on_pool else nc.vector
                        eng.tensor_tensor(
                            eyt[:],
                            t[:, 2:P + 2, c0 + 1:c0 + 65],
                            t[:, 0:P, c0 + 1:c0 + 65],
                            op=Alu.subtract,
                        )
                        eyts[q] = eyt
                for q in range(2):
                    c0 = 64 * q
                    defer_ex = (g == 0 and q < dq0) or (
                        dq_last and s == ns - 1)
                    if defer_ex:
                        # defer: Pool subtract + ACT accs emitted after
                        # the g1 unit (see emit loop). The ext tile MUST
                        # be allocated at emission time, not now, or the
                        # pool recycles its slot in between.
                        def _emit(t=t, c0=c0, q=q, exs=exs):
                            ext = ex_pool.tile([128, P, 64], f32,
                                               tag="ext")
                            nc.gpsimd.tensor_tensor(
                                ext[:],
                                t[:, 1:P + 1, c0 + 2:c0 + 66],
                                t[:, 1:P + 1, c0:c0 + 64],
                                op=Alu.subtract,
                            )
                            for hh in range(2):
                                absacc_act(
                                    ext[:, :, P * hh:P * hh + P],
                                    exs[:, 2 * q + hh:2 * q + hh + 1])
                        deferred.append(_emit)
                        skip_ex_acc = True
                    else:
                        skip_ex_acc = False
                        ext = ex_pool.tile([128, P, 64], f32, tag="ext")
                        ex_eng = nc.gpsimd if q < tq else nc.vector
                        ex_eng.tensor_tensor(
                            ext[:],
                            t[:, 1:P + 1, c0 + 2:c0 + 66],
                            t[:, 1:P + 1, c0:c0 + 64],
                            op=Alu.subtract,
                        )
                    if ey_first:
                        eyt = eyts[q]
                    else:
                        eyt = ey_pool.tile([128, P, 64], f32, tag="eyt")
                        gqg = gq if (g == 0 or gq1 is None) else gq1
                        on_pool = q < gqg and not (s == 0 and s0_dve)
                        eng = nc.gpsimd if on_pool else nc.vector
                        eng.tensor_tensor(
                            eyt[:],
                            t[:, 2:P + 2, c0 + 1:c0 + 65],
                            t[:, 0:P, c0 + 1:c0 + 65],
                            op=Alu.subtract,
                        )
                    if not skip_ex_acc:
                        if q < exd:
                            absacc_mp(ext[:], 2, 64, exs[:, 2 * q:2 * q + 2])
                        else:
                            for hh in range(2):
                                absacc_act(
                                    ext[:, :, P * hh:P * hh + P],
                                    exs[:, 2 * q + hh:2 * q + hh + 1])
                    # optionally defer ey-accs behind the ex/x accs so the
                    # ACT queue never head-of-line blocks on Pool's ey
                    def _eyacc(eyt=eyt, q=q):
                        if q < eyd:
                            absacc_mp(eyt[:], 2, 64, eys[:, 2 * q:2 * q + 2])
                        else:
                            for hh in range(2):
                                absacc_act(eyt[:, :, P * hh:P * hh + P],
                                           eys[:, 2 * q + hh:2 * q + hh + 1])
                    if eyorder:
                        ey_accs.append(_eyacc)
                    else:
                        _eyacc()
                if xd:
                    absacc_mp(t[:, 1:P + 1, 1:1 + 4 * P], 4, hw_ + 2,
                              xs_[:, 0:4])
                else:
                    for pu in range(4):
                        absacc_act(t[:, 1:P + 1, 1 + P * pu:1 + P * (pu + 1)],
                                   xs_[:, pu:pu + 1])
                for f in ey_accs:
                    f()
                return t, (exs, eys, xs_), deferred

            def emit_sp(h, g, accs):
                """sp = exs + eys + xs (ranks invariant to positive
                scale). Emitted after any deferred accs."""
                exs, eys, xs_ = accs
                spg = acc_pool.tile([128, nph], f32, tag="sp",
                                    name=f"sp{g}{h}")
                nc.vector.scalar_tensor_tensor(
                    spg[:], exs[:], 1.0, eys[:],
                    op0=Alu.mult, op1=Alu.add,
                )
                nc.vector.tensor_tensor(spg[:], spg[:], xs_[:], op=Alu.add)
                return spg

            def emit_rank_store(s, h, xt, sp):
                """Rank the nph patches of (s, h) and mask + store both
                groups. xt/sp: per-group x tiles and sp accums."""
                from contextlib import nullcontext
                r0 = s * P
                hi = tc.high_priority() if rank_hi else nullcontext()
                with hi:
                    _rank_body(s, h, xt, sp, r0)

            def _rank_body(s, h, xt, sp, r0):
                nsp = []
                if not sign_dve:
                    for g in range(2):
                        nspg = acc_pool.tile([128, nph], f32, tag="nsp",
                                             name=f"nsp{g}{h}")
                        nc.vector.tensor_scalar(
                            nspg[:], sp[g][:], -1.0, None, op0=Alu.mult
                        )
                        nsp.append(nspg)

                # transpose sp -> spT (nph, 256): patches x channels
                spT = spt_pool.tile([nph, 256], f32, tag="spT")
                for g in range(2):
                    pt = ps_t.tile([nph, 128], f32, tag="psT")
                    nc.tensor.transpose(pt[:], sp[g][:], ident[:])
                    nc.vector.tensor_copy(
                        spT[:, g * 128:(g + 1) * 128], pt[:]
                    )

                # per patch: broadcast spT row to 128 partitions and
                # count strictly-greater channels
                sgn = [
                    acc_pool.tile([128, nph], f32, tag="sgn",
                                  name=f"sgn{g}{h}")
                    for g in range(2)
                ]
                for p in range(nph):
                    pb = ps_b.tile([128, 256], f32, tag="pb")
                    nc.tensor.matmul(
                        pb[:], onehot[:, 128 * p:128 * (p + 1)], spT[:],
                        start=True, stop=True,
                    )
                    for g in range(2):
                        if sign_dve:
                            # cnt[c] = #(c' with sp[c'] > sp[c])
                            nc.vector.tensor_scalar(
                                dve_dump2, pb[:], sp[g][:, p:p + 1],
                                None,
                                op0=Alu.is_gt, op1=Alu.add,
                                accum_out=sgn[g][:, p:p + 1],
                            )
                        else:
                            # sgn[c] = #gt - #lt  (= 2*cnt - 255)
                            po = ps_sgn.tile([128, 256], f32, tag="po")
                            nc.scalar.activation(
                                po[:], pb[:], Act.Sign,
                                bias=nsp[g][:, p:p + 1],
                                accum_out=sgn[g][:, p:p + 1],
                            )

                # keep iff fewer than 128 strictly greater
                thresh = 127.5 if sign_dve else -0.5
                mask = []
                for g in range(2):
                    mg = acc_pool.tile([128, nph], f32, tag="mask",
                                       name=f"mask{g}{h}")
                    nc.vector.tensor_scalar(
                        mg[:], sgn[g][:], thresh, None, op0=Alu.is_le
                    )
                    mask.append(mg)

                # apply mask and store (out DMA on the ACT HWDGE queue
                # right after this unit's ACT maskmuls). With stage=1 the
                # masked copy goes to a staging tile so the x tile frees
                # at maskmul time, not out-DMA completion.
                ms = msplit
                if msplit_last is not None and s >= ns - 1 - msl2:
                    # drain: split the last strip's maskmuls across DVE
                    # and ACT -- ACT's queue is empty by then
                    ms = msplit_last
                for g in range(2):
                    t = xt[g]
                    if stage:
                        st = st_pool.tile([128, P, hw_], f32, tag="st")
                    for pu in range(nph):
                        reg = t[:, 1:P + 1, 1 + P * pu:1 + P * (pu + 1)]
                        dst = (st[:, :, P * pu:P * (pu + 1)]
                               if stage else reg)
                        if pu < ms:
                            nc.vector.tensor_scalar(
                                dst, reg, mask[g][:, pu:pu + 1], None,
                                op0=Alu.mult,
                            )
                        else:
                            nc.scalar.activation(
                                dst, reg, Act.Copy,
                                scale=mask[g][:, pu:pu + 1],
                            )
                    def _issue(g=g, st=(st if stage else None), t=t):
                        if osplit:
                            for rr in range(2):
                                nc.scalar.dma_start(
                                    o_ap[g * 128:(g + 1) * 128,
                                         r0 + 16 * rr:r0 + 16 * (rr + 1),
                                         h * hw_:(h + 1) * hw_],
                                    st[:, 16 * rr:16 * (rr + 1), :]
                                    if stage else
                                    t[:, 1 + 16 * rr:1 + 16 * (rr + 1),
                                      1:hw_ + 1],
                                )
                        else:
                            nc.scalar.dma_start(
                                o_ap[g * 128:(g + 1) * 128, r0:r0 + P,
                                     h * hw_:(h + 1) * hw_],
                                st[:] if stage else t[:, 1:P + 1, 1:hw_ + 1],
                            )
                    if out_defer:
                        pending_outs.append(_issue)
                    else:
                        _issue()

            pending_outs = []
            for s in range(ns):
                for h in range(2):
                    xt = {}
                    acc = {}
                    sp = {}
                    defs = []
                    for g in range(2):
                        xt[g], acc[g], dd = emit_unit(s, h, g)
                        defs.extend(dd)
                    for d in defs:
                        d()
                    for g in range(2):
                        sp[g] = emit_sp(h, g, acc[g])
                    prev = list(pending_outs)
                    pending_outs.clear()
                    emit_rank_store(s, h, xt, sp)
                    for f in prev:
                        f()
            for f in pending_outs:
                f()

    nc.compile()
    return nc


def onehot_input(nph=4):
    oh = np.zeros((nph, nph * 128), np.float32)
    for p in range(nph):
        oh[p, 128 * p:128 * (p + 1)] = 1.0
    return oh


BEST = dict(xbufs=8, gq=2, tq0=1, tq1=0, dq0=0, exd0=0, exd1=0, eyd0=0,
            eyd1=1, xd0=0, xd1=1, msplit=4, sign_dve=True, stage=0,
            stbufs=2, s0_dve=0, out_defer=1, eybufs=3, msplit_last=3,
            tq_s0=0)


def _get_program():
    key = "full"
    if key not in _cache:
        _cache[key] = build_program(**BEST)
    return _cache[key]


def kernel(x):
    """x: (4, 256, 256, 256) float32 -> masked output, same shape."""
    from concourse.bass_utils import run_bass_kernel_spmd

    x = np.asarray(x)
    assert x.shape == (N_VEH, C, H, W) and x.dtype == np.float32

    nc = _get_program()

    xp = np.pad(x, ((0, 0), (0, 0), (1, 1), (1, 1)))
    oh = onehot_input()
    in_maps = []
    for n in range(N_VEH):
        for hh in range(2):
            shard = np.ascontiguousarray(xp[n, :, hh * HS:hh * HS + HS + 2, :])
            in_maps.append({"x": shard, "onehot": oh})

    res = run_bass_kernel_spmd(nc, in_maps, list(range(N_CORES)))

    out = np.empty((N_VEH, C, H, W), np.float32)
    for n in range(N_VEH):
        for hh in range(2):
            out[n, :, hh * HS:hh * HS + HS, :] = res.results[n * 2 + hh]["out"]
    return out



# revision 2
# speedup vs baseline: 39965.2792x; 39965.2792x over previous
"""Trainium2 Bass kernel for ChannelSelection (top-k channel masking).

Reference computation (per vehicle n of N=4):
  s = 0.5*grad_mag(x) + 0.5*|x|            # grad_mag = |x[w+1]-x[w-1]| + |x[h+1]-x[h-1]|
  sp[c, patch] = mean of s over 32x32 patch
  keep top-128 (of 256) channels per patch (rank by sp desc, stable)
  out = x * mask broadcast over patch

Sharding: 8 cores = N(4) x H-halves(2). Each core gets one vehicle's
channel-complete slab of 128 rows (+1 halo row each side, zero padded
by the host) and computes its patches' top-k independently (patches
never straddle the H split since 128 % 32 == 0).

Structure (per core): 16 units = 4 row-strips x 2 column-halves x 2
channel-groups, each a [128, 34, 130] SBUF tile (1px halo baked in),
ring of 8. Units stream load -> grad/abs-accumulate; after both channel
groups of a (strip, half) land, that half's 4 patches are ranked and
masked in place -> store. Input DMAs issue on the SP HWDGE queue,
output DMAs on the ACT HWDGE queue, deferred by one half so the ACT
sequencer never stalls waiting for DVE maskmuls.

Engine balance (per core, cost-model ns/elem; only ISA-legal ops --
tensor_scalar(abs_max) and Pool accumulations are rejected by the
NEFF compiler):
  DVE  : ex = x[w+1]-x[w-1] TT; |x| patch sums as one 4-patch
         tensor_reduce(abs) per g1 unit + one 2-patch ey reduce;
         rank counts via is_gt+accum vs the PE broadcast; maskmul via
         2x tensor_scalar mult (in place)
  ACT  : per-patch Abs+accum of ex/ey (and g0 |x|)
  GPSIMD: ey = x[h+1]-x[h-1] TT
  PE   : transpose sp + one-hot broadcast matmul for the rank compare
  DMA floor ~194us is the target bottleneck (in 36.2MB + out 33.5MB
  per core at ~360 GB/s).
"""

import dataclasses
import os
import sys

import numpy as np

_TRN_REPO = "/opt/trn_rl_repo"
if _TRN_REPO not in sys.path:
    sys.path.insert(0, _TRN_REPO)

# Full-problem constants (hardcoded per contest rules)
N_VEH = 4
C = 256
H = 256
W = 256
P = 32          # patch size
N_CORES = 8
HS = 128        # rows per core (H/2)

_cache = {}


def build_program(ns=4, npc=8, xbufs=7, gq=2, gq1=None, tq0=0, tq1=0, dq0=1,
                  exd0=0, exd1=0, eyd0=0, eyd1=1, xd0=0, xd1=1,
                  msplit=4, sign_dve=True, stage=1, stbufs=2, s0_dve=0,
                  out_defer=1, exbufs=2, eybufs=2, eyorder=0, rank_hi=0, dq_last=0, ey_first=0, s3_act=0, osplit=0, msplit_last=None, msl2=0, tq_s0=None, tq_s0_span=0, reps=1):
    """Build the SPMD Bass program for one core.

    ns:      number of 32-row strips (ns*32 = rows per core)
    npc:     number of patch columns (npc*32 = W)
    xbufs:   x-tile ring depth (units in flight)
    gq:      ey-subtract quarters per unit on GPSIMD, rest DVE (0..2)
    tq0/tq1: ex-subtract quarters per unit on GPSIMD by group (0..2)
    dq0:     deferred GPSIMD ex-quarters for the g0 unit of each half,
             emitted after the g1 unit so Pool lag misses the barrier
    exd0/1:  ex abs-acc quarters per unit on DVE (2-patch reduce) by
             group, rest ACT per-patch Abs+accum (0..2)
    eyd0/1:  same for ey abs-acc
    xd0/1:   |x| abs-acc on DVE as one 4-patch reduce (1) or ACT (0)
    msplit:  maskmul patches per unit on DVE (2x tensor_scalar), rest
             ACT Copy+scale (0..4)
    sign_dve: rank count via DVE is_gt+accum instead of ACT Sign+accum
    stage:   maskmul writes a separate staging tile (frees x at maskmul
             time instead of out-DMA completion); 0 = in-place
    stbufs:  staging ring depth
    s0_dve:  strip 0 runs its ey subs on DVE (fill-phase fast track)
    reps:    repeat the whole pipeline (timing harness only)
    """
    from contextlib import ExitStack

    import concourse.bass as bass
    import concourse.tile as tile
    from concourse import bacc, masks, mybir

    f32 = mybir.dt.float32
    Alu = mybir.AluOpType
    Act = mybir.ActivationFunctionType

    rows = ns * P
    w = npc * P
    wp = w + 2
    hw_ = w // 2          # out cols per half (128)
    nph = npc // 2        # patches per half (4)

    nc = bacc.Bacc("TRN2", target_bir_lowering=False, debug=False)
    x_ap = nc.dram_tensor("x", [C, rows + 2, wp], f32, kind="ExternalInput").ap()
    oh_ap = nc.dram_tensor(
        "onehot", [nph, nph * 128], f32, kind="ExternalInput"
    ).ap()
    o_ap = nc.dram_tensor("out", [C, rows, w], f32, kind="ExternalOutput").ap()

    with tile.TileContext(nc) as tc, ExitStack() as ctx:
        const_pool = ctx.enter_context(tc.tile_pool(name="const", bufs=1))
        x_pool = ctx.enter_context(tc.tile_pool(name="xs", bufs=xbufs))
        ex_pool = ctx.enter_context(tc.tile_pool(name="ex", bufs=exbufs))
        ey_pool = ctx.enter_context(tc.tile_pool(name="ey", bufs=eybufs))
        if stage:
            # out_defer keeps a staging tile alive across the next half's
            # maskmuls -- need at least 4 slots to avoid aliasing
            if out_defer:
                stbufs = max(stbufs, 4)
            st_pool = ctx.enter_context(tc.tile_pool(name="st", bufs=stbufs))
        acc_pool = ctx.enter_context(tc.tile_pool(name="acc", bufs=16))
        spt_pool = ctx.enter_context(tc.tile_pool(name="spt", bufs=3))
        scr_pool = ctx.enter_context(tc.tile_pool(name="scr", bufs=1))
        ps_b = ctx.enter_context(tc.tile_pool(name="ps_b", bufs=5, space="PSUM"))
        ps_sgn = ctx.enter_context(tc.tile_pool(name="ps_sgn", bufs=1, space="PSUM"))
        ps_t = ctx.enter_context(tc.tile_pool(name="ps_t", bufs=2, space="PSUM"))

        # constants
        ident = const_pool.tile([128, 128], f32)
        masks.make_identity(nc, ident[:])
        # onehot[p, 128p:128(p+1)] = 1: stationary for the patch-row broadcast
        onehot = const_pool.tile([nph, nph * 128], f32)
        nc.scalar.dma_start(onehot[:], oh_ap[:])

        # engine-private scratch (mandatory elementwise outputs, discarded)
        dve_dump = scr_pool.tile([128, P, P], f32, tag="dve_dump")
        act_dump = scr_pool.tile([128, P, P], f32, tag="act_dump")
        _dd = dve_dump[:, :, :]
        dve_dump2 = dataclasses.replace(_dd, ap=[_dd.ap[0], [1, 256]])

        def absacc_act(src, accum):
            """accum[:,0] = per-partition sum |src| on ACT."""
            nc.scalar.activation(act_dump[:], src, Act.Abs, accum_out=accum)

        def absacc_mp(src, np_, row_stride, accum):
            """accum[:, 0:np_] = per-patch sum |src| via one DVE
            tensor_reduce over a [128, np_, P, P] view of src (src is a
            [128, P, np_*P] region with the given row stride)."""
            v = dataclasses.replace(
                src, ap=[src.ap[0], [P, np_], [row_stride, P], [1, P]]
            )
            nc.vector.tensor_reduce(
                accum, v, axis=mybir.AxisListType.XY, op=Alu.add,
                apply_absolute_value=True,
            )

        for rep in range(reps):

            def emit_unit(s, h, g):
                """Load + grad + abs-accum for unit (strip s, half h,
                group g). Returns (x tile, sp accum [128, nph])."""
                r0 = s * P
                t = x_pool.tile([128, P + 2, hw_ + 2], f32, tag="x",
                                name=f"x{g}{h}")
                nc.sync.dma_start(
                    t[:],
                    x_ap[g * 128:(g + 1) * 128, r0:r0 + P + 2,
                         h * hw_:h * hw_ + hw_ + 2],
                )
                exs = acc_pool.tile([128, nph], f32, tag="exs",
                                    name=f"exs{g}{h}")
                eys = acc_pool.tile([128, nph], f32, tag="eys",
                                    name=f"eys{g}{h}")
                xs_ = acc_pool.tile([128, nph], f32, tag="xs",
                                    name=f"xs{g}{h}")
                tq = (tq0, tq1)[g]
                if tq_s0 is not None and s <= tq_s0_span:
                    # fill: Pool starts cold; keep strip 0's ex on DVE
                    tq = tq_s0
                exd = (exd0, exd1)[g]
                eyd = (eyd0, eyd1)[g]
                xd = (xd0, xd1)[g]
                if s3_act and s == ns - 1:
                    # drain: DVE's queue tail is the longest; push the
                    # last strip's reduce-accs to ACT instead
                    eyd = 0
                    xd = 0
                deferred = []
                ey_accs = []
                eyts = {}
                if ey_first:
                    # emit both ey subtracts first so a Pool-assigned ex
                    # quarter doesn't delay the ey chain
                    for q in range(2):
                        c0 = 64 * q
                        eyt = ey_pool.tile([128, P, 64], f32, tag="eyt")
                        gqg = gq if (g == 0 or gq1 is None) else gq1
                        on_pool = q < gqg and not (s == 0 and s0_dve)
                        eng = nc.gpsimd if on_pool else nc.vector
                        eng.tensor_tensor(
                            eyt[:],
                            t[:, 2:P + 2, c0 + 1:c0 + 65],
                            t[:, 0:P, c0 + 1:c0 + 65],
                            op=Alu.subtract,
                        )
                        eyts[q] = eyt
                for q in range(2):
                    c0 = 64 * q
                    defer_ex = (g == 0 and q < dq0) or (
                        dq_last and s == ns - 1)
                    if defer_ex:
                        # defer: Pool subtract + ACT accs emitted after
                        # the g1 unit (see emit loop). The ext tile MUST
                        # be allocated at emission time, not now, or the
                        # pool recycles its slot in between.
                        def _emit(t=t, c0=c0, q=q, exs=exs):
                            ext = ex_pool.tile([128, P, 64], f32,
                                               tag="ext")
                            nc.gpsimd.tensor_tensor(
                                ext[:],
                                t[:, 1:P + 1, c0 + 2:c0 + 66],
                                t[:, 1:P + 1, c0:c0 + 64],
                                op=Alu.subtract,
                            )
                            for hh in range(2):
                                absacc_act(
                                    ext[:, :, P * hh:P * hh + P],
                                    exs[:, 2 * q + hh:2 * q + hh + 1])
                        deferred.append(_emit)
                        skip_ex_acc = True
                    else:
                        skip_ex_acc = False
                        ext = ex_pool.tile([128, P, 64], f32, tag="ext")
                        ex_eng = nc.gpsimd if q < tq else nc.vector
                        ex_eng.tensor_tensor(
                            ext[:],
                            t[:, 1:P + 1, c0 + 2:c0 + 66],
                            t[:, 1:P + 1, c0:c0 + 64],
                            op=Alu.subtract,
                        )
                    if ey_first:
                        eyt = eyts[q]
                    else:
                        eyt = ey_pool.tile([128, P, 64], f32, tag="eyt")
                        gqg = gq if (g == 0 or gq1 is None) else gq1
                        on_pool = q < gqg and not (s == 0 and s0_dve)
                        eng = nc.gpsimd if on_pool else nc.vector
                        eng.tensor_tensor(
                            eyt[:],
                            t[:, 2:P + 2, c0 + 1:c0 + 65],
                            t[:, 0:P, c0 + 1:c0 + 65],
                            op=Alu.subtract,
                        )
                    if not skip_ex_acc:
                        if q < exd:
                            absacc_mp(ext[:], 2, 64, exs[:, 2 * q:2 * q + 2])
                        else:
                            for hh in range(2):
                                absacc_act(
                                    ext[:, :, P * hh:P * hh + P],
                                    exs[:, 2 * q + hh:2 * q + hh + 1])
                    # optionally defer ey-accs behind the ex/x accs so the
                    # ACT queue never head-of-line blocks on Pool's ey
                    def _eyacc(eyt=eyt, q=q):
                        if q < eyd:
                            absacc_mp(eyt[:], 2, 64, eys[:, 2 * q:2 * q + 2])
                        else:
                            for hh in range(2):
                                absacc_act(eyt[:, :, P * hh:P * hh + P],
                                           eys[:, 2 * q + hh:2 * q + hh + 1])
                    if eyorder:
                        ey_accs.append(_eyacc)
                    else:
                        _eyacc()
                if xd:
                    absacc_mp(t[:, 1:P + 1, 1:1 + 4 * P], 4, hw_ + 2,
                              xs_[:, 0:4])
                else:
                    for pu in range(4):
                        absacc_act(t[:, 1:P + 1, 1 + P * pu:1 + P * (pu + 1)],
                                   xs_[:, pu:pu + 1])
                for f in ey_accs:
                    f()
                return t, (exs, eys, xs_), deferred

            def emit_sp(h, g, accs):
                """sp = exs + eys + xs (ranks invariant to positive
                scale). Emitted after any deferred accs."""
                exs, eys, xs_ = accs
                spg = acc_pool.tile([128, nph], f32, tag="sp",
                                    name=f"sp{g}{h}")
                nc.vector.scalar_tensor_tensor(
                    spg[:], exs[:], 1.0, eys[:],
                    op0=Alu.mult, op1=Alu.add,
                )
                nc.vector.tensor_tensor(spg[:], spg[:], xs_[:], op=Alu.add)
                return spg

            def emit_rank_store(s, h, xt, sp):
                """Rank the nph patches of (s, h) and mask + store both
                groups. xt/sp: per-group x tiles and sp accums."""
                from contextlib import nullcontext
                r0 = s * P
                hi = tc.high_priority() if rank_hi else nullcontext()
                with hi:
                    _rank_body(s, h, xt, sp, r0)

            def _rank_body(s, h, xt, sp, r0):
                nsp = []
                if not sign_dve:
                    for g in range(2):
                        nspg = acc_pool.tile([128, nph], f32, tag="nsp",
                                             name=f"nsp{g}{h}")
                        nc.vector.tensor_scalar(
                            nspg[:], sp[g][:], -1.0, None, op0=Alu.mult
                        )
                        nsp.append(nspg)

                # transpose sp -> spT (nph, 256): patches x channels
                spT = spt_pool.tile([nph, 256], f32, tag="spT")
                for g in range(2):
                    pt = ps_t.tile([nph, 128], f32, tag="psT")
                    nc.tensor.transpose(pt[:], sp[g][:], ident[:])
                    nc.vector.tensor_copy(
                        spT[:, g * 128:(g + 1) * 128], pt[:]
                    )

                # per patch: broadcast spT row to 128 partitions and
                # count strictly-greater channels
                sgn = [
                    acc_pool.tile([128, nph], f32, tag="sgn",
                                  name=f"sgn{g}{h}")
                    for g in range(2)
                ]
                for p in range(nph):
                    pb = ps_b.tile([128, 256], f32, tag="pb")
                    nc.tensor.matmul(
                        pb[:], onehot[:, 128 * p:128 * (p + 1)], spT[:],
                        start=True, stop=True,
                    )
                    for g in range(2):
                        if sign_dve:
                            # cnt[c] = #(c' with sp[c'] > sp[c])
                            nc.vector.tensor_scalar(
                                dve_dump2, pb[:], sp[g][:, p:p + 1],
                                None,
                                op0=Alu.is_gt, op1=Alu.add,
                                accum_out=sgn[g][:, p:p + 1],
                            )
                        else:
                            # sgn[c] = #gt - #lt  (= 2*cnt - 255)
                            po = ps_sgn.tile([128, 256], f32, tag="po")
                            nc.scalar.activation(
                                po[:], pb[:], Act.Sign,
                                bias=nsp[g][:, p:p + 1],
                                accum_out=sgn[g][:, p:p + 1],
                            )

                # keep iff fewer than 128 strictly greater
                thresh = 127.5 if sign_dve else -0.5
                mask = []
                for g in range(2):
                    mg = acc_pool.tile([128, nph], f32, tag="mask",
                                       name=f"mask{g}{h}")
                    nc.vector.tensor_scalar(
                        mg[:], sgn[g][:], thresh, None, op0=Alu.is_le
                    )
                    mask.append(mg)

                # apply mask and store (out DMA on the ACT HWDGE queue
                # right after this unit's ACT maskmuls). With stage=1 the
                # masked copy goes to a staging tile so the x tile frees
                # at maskmul time, not out-DMA completion.
                ms = msplit
                if msplit_last is not None and s >= ns - 1 - msl2:
                    # drain: split the last strip's maskmuls across DVE
                    # and ACT -- ACT's queue is empty by then
                    ms = msplit_last
                for g in range(2):
                    t = xt[g]
                    if stage:
                        st = st_pool.tile([128, P, hw_], f32, tag="st")
                    for pu in range(nph):
                        reg = t[:, 1:P + 1, 1 + P * pu:1 + P * (pu + 1)]
                        dst = (st[:, :, P * pu:P * (pu + 1)]
                               if stage else reg)
                        if pu < ms:
                            nc.vector.tensor_scalar(
                                dst, reg, mask[g][:, pu:pu + 1], None,
                                op0=Alu.mult,
                            )
                        else:
                            nc.scalar.activation(
                                dst, reg, Act.Copy,
                                scale=mask[g][:, pu:pu + 1],
                            )
                    def _issue(g=g, st=(st if stage else None), t=t):
                        if osplit:
                            for rr in range(2):
                                nc.scalar.dma_start(
                                    o_ap[g * 128:(g + 1) * 128,
                                         r0 + 16 * rr:r0 + 16 * (rr + 1),
                                         h * hw_:(h + 1) * hw_],
                                    st[:, 16 * rr:16 * (rr + 1), :]
                                    if stage else
                                    t[:, 1 + 16 * rr:1 + 16 * (rr + 1),
                                      1:hw_ + 1],
                                )
                        else:
                            nc.scalar.dma_start(
                                o_ap[g * 128:(g + 1) * 128, r0:r0 + P,
                                     h * hw_:(h + 1) * hw_],
                                st[:] if stage else t[:, 1:P + 1, 1:hw_ + 1],
                            )
                    if out_defer:
                        pending_outs.append(_issue)
                    else:
                        _issue()

            pending_outs = []
            for s in range(ns):
                for h in range(2):
                    xt = {}
                    acc = {}
                    sp = {}
                    defs = []
                    for g in range(2):
                        xt[g], acc[g], dd = emit_unit(s, h, g)
                        defs.extend(dd)
                    for d in defs:
                        d()
                    for g in range(2):
                        sp[g] = emit_sp(h, g, acc[g])
                    prev = list(pending_outs)
                    pending_outs.clear()
                    emit_rank_store(s, h, xt, sp)
                    for f in prev:
                        f()
            for f in pending_outs:
                f()

    nc.compile()
    return nc


def onehot_input(nph=4):
    oh = np.zeros((nph, nph * 128), np.float32)
    for p in range(nph):
        oh[p, 128 * p:128 * (p + 1)] = 1.0
    return oh


BEST = dict(xbufs=8, gq=2, tq0=1, tq1=0, dq0=0, exd0=0, exd1=0, eyd0=0,
            eyd1=1, xd0=0, xd1=1, msplit=4, sign_dve=True, stage=0,
            stbufs=2, s0_dve=0, out_defer=1, eybufs=3, msplit_last=3,
            tq_s0=0)


def _get_program():
    key = "full"
    if key not in _cache:
        _cache[key] = build_program(**BEST)
    return _cache[key]


def make_in_maps(x):
    """Build the 8 per-core input maps from the full (4,256,256,256) x."""
    xp = np.pad(x, ((0, 0), (0, 0), (1, 1), (1, 1)))
    oh = onehot_input()
    in_maps = []
    for n in range(N_VEH):
        for hh in range(2):
            shard = np.ascontiguousarray(xp[n, :, hh * HS:hh * HS + HS + 2, :])
            in_maps.append({"x": shard, "onehot": oh})
    return in_maps


def kernel(x):
    """x: (4, 256, 256, 256) float32 -> masked output, same shape."""
    from concourse.bass_utils import run_bass_kernel_spmd

    x = np.asarray(x)
    assert x.shape == (N_VEH, C, H, W) and x.dtype == np.float32

    nc = _get_program()
    res = run_bass_kernel_spmd(nc, make_in_maps(x), list(range(N_CORES)))

    out = np.empty((N_VEH, C, H, W), np.float32)
    for n in range(N_VEH):
        for hh in range(2):
            out[n, :, hh * HS:hh * HS + HS, :] = res.results[n * 2 + hh]["out"]
    return out



# revision 3
# speedup vs baseline: 46289.6609x; 1.1582x over previous
"""Trainium2 Bass kernel v2 for ChannelSelection (top-k channel masking).

Reference computation (per vehicle n of N=4):
  s = 0.5*grad_mag(x) + 0.5*|x|            # grad_mag = |x[w+1]-x[w-1]| + |x[h+1]-x[h-1]|
  sp[c, patch] = mean of s over 32x32 patch
  keep top-128 (of 256) channels per patch (rank by sp desc)
  out = x * mask broadcast over patch

Differences vs v1 (378us): the host pre-splits each core's slab into two
width-halves with their own 1-px halo, stored contiguously, so every
input DMA is one ~17.7KB contiguous span per partition (one descriptor)
instead of 34 strided 520B rows; output is fp16 (rel err ~2e-4, within
the 2e-2 gate) and also lands as one 8KB span per partition. Subtracts
run as flat 1-D contiguous tensor_tensor over whole tiles (halo columns
produce garbage lanes that the patch-sum views never read).

Sharding: 8 cores = N(4) x H-halves(2). Per core: 16 units =
4 row-strips x 2 width-halves x 2 channel-groups, tile [128, 34, 130].

Engine split (knobs in BEST): DVE ex-subtract + |ey| patch-reduce +
maskmul; ACT |ex| + |x| patch-accumulate; Pool ey-subtract; PE rank
broadcast. Rank counts split DVE(is_gt)/ACT(Sign).
"""

import dataclasses
import sys

import numpy as np

_TRN_REPO = "/opt/trn_rl_repo"
if _TRN_REPO not in sys.path:
    sys.path.insert(0, _TRN_REPO)

N_VEH = 4
C = 256
H = 256
W = 256
P = 32
N_CORES = 8
HS = 128          # rows per core
NPH = 4           # patches per width-half
HW = 128          # out cols per half
WIN = 130         # in cols per half (1px halo each side)

_cache = {}


def build_program(xbufs=4, scrbufs=3, stbufs=3,
                  ex_eng=(0, 0), ey_eng=(1, 1),
                  accx_eng=(0, 0), accex_eng=(0, 0), accey_eng=(1, 1),
                  mask_eng=(1, 0), cnt_eng=(1, 0),
                  out_defer=1, ey_s0_dve=1, sp_pool=0, scr_f16=0):
    """One-core SPMD program.

    Engine codes, per channel-group g: subtracts 0=DVE 1=Pool;
    accs 0=ACT(per-patch activation Abs) 1=DVE(tensor_reduce XY abs);
    mask_eng 0=ACT(Copy scale) 1=DVE(tensor_scalar mult);
    cnt_eng 0=ACT(Sign bias accum) 1=DVE(is_gt accum).
    ey_s0_dve: strip-0 ey subs forced to DVE (Pool starts cold).
    """
    from contextlib import ExitStack

    import concourse.bass as bass
    import concourse.tile as tile
    from concourse import bacc, masks, mybir

    f32 = mybir.dt.float32
    f16 = mybir.dt.float16
    Alu = mybir.AluOpType
    Act = mybir.ActivationFunctionType

    ns = 4                      # strips
    rows = ns * P               # 128

    nc = bacc.Bacc("TRN2", target_bir_lowering=False, debug=False)
    # [half, ch] flattened: per-partition rows are contiguous spans
    x_ap = nc.dram_tensor("x", [2 * C, rows + 2, WIN], f32,
                          kind="ExternalInput").ap()
    oh_ap = nc.dram_tensor("onehot", [NPH, NPH * 128], f32,
                           kind="ExternalInput").ap()
    o_ap = nc.dram_tensor("out", [2 * C, rows, HW], f16,
                          kind="ExternalOutput").ap()

    with tile.TileContext(nc) as tc, ExitStack() as ctx:
        const_pool = ctx.enter_context(tc.tile_pool(name="const", bufs=1))
        x_pool = ctx.enter_context(tc.tile_pool(name="xs", bufs=xbufs))
        scr_pool = ctx.enter_context(tc.tile_pool(name="scr", bufs=scrbufs))
        st_pool = ctx.enter_context(tc.tile_pool(name="st", bufs=stbufs))
        acc_pool = ctx.enter_context(tc.tile_pool(name="acc", bufs=22))
        spt_pool = ctx.enter_context(tc.tile_pool(name="spt", bufs=2))
        dump_pool = ctx.enter_context(tc.tile_pool(name="dump", bufs=1))
        ps_b = ctx.enter_context(tc.tile_pool(name="ps_b", bufs=5, space="PSUM"))
        ps_sgn = ctx.enter_context(tc.tile_pool(name="ps_sgn", bufs=1, space="PSUM"))
        ps_t = ctx.enter_context(tc.tile_pool(name="ps_t", bufs=2, space="PSUM"))

        ident = const_pool.tile([128, 128], f32)
        masks.make_identity(nc, ident[:])
        onehot = const_pool.tile([NPH, NPH * 128], f32)
        nc.scalar.dma_start(onehot[:], oh_ap[:])

        dve_dump = dump_pool.tile([128, 256], f16, tag="dve_dump")
        act_dump = dump_pool.tile([128, P, P], f16, tag="act_dump")
        act_dump3 = act_dump[:, :, :]
        _dd = dve_dump[:, :]
        dve_dump2 = dataclasses.replace(_dd, ap=[_dd.ap[0], [1, 256]])

        def flat(ap3, offset, count):
            """Flat 1-D view of a [128, R, Ccols] tile AP from elem offset."""
            r0, c0 = divmod(offset, ap3.shape[2])
            sl = ap3[:, r0:, c0:]
            return dataclasses.replace(sl, ap=[sl.ap[0], [1, count]])

        def absacc_act(src3, accum):
            nc.scalar.activation(act_dump3, src3, Act.Abs, accum_out=accum)

        def absacc_dve(src, np_, row_stride, accum):
            v = dataclasses.replace(
                src, ap=[src.ap[0], [P, np_], [row_stride, P], [1, P]]
            )
            nc.vector.tensor_reduce(
                accum, v, axis=mybir.AxisListType.XY, op=Alu.add,
                apply_absolute_value=True,
            )

        def emit_unit(s, h, g):
            """Load + subtracts + patch-accumulates for one unit."""
            r0 = s * P
            part0 = h * C + g * 128
            t = x_pool.tile([128, P + 2, WIN], f32, tag="x", name=f"x{g}{h}")
            nc.sync.dma_start(t[:], x_ap[part0:part0 + 128, r0:r0 + P + 2, :])

            exs = acc_pool.tile([128, NPH], f32, tag="exs", name=f"exs{g}{h}")
            eys = acc_pool.tile([128, NPH], f32, tag="eys", name=f"eys{g}{h}")
            xs_ = acc_pool.tile([128, NPH], f32, tag="xs", name=f"xs{g}{h}")

            t3 = t[:, :, :]
            K = P * WIN  # 4160
            sdt = f16 if scr_f16 else f32

            # ex: ext[q, i] = t[q+1, i+2] - t[q+1, i]  (valid out-col i<=127)
            ext = scr_pool.tile([128, P, WIN], sdt, tag="ext")
            eng = nc.gpsimd if ex_eng[g] else nc.vector
            eng.tensor_tensor(
                flat(ext[:, :, :], 0, K),
                flat(t3, WIN + 2, K), flat(t3, WIN, K),
                op=Alu.subtract,
            )
            # |ex| patch sums
            if accex_eng[g]:
                absacc_dve(ext[:, 0:P, 0:NPH * P], NPH, WIN, exs[:, 0:NPH])
            else:
                for p in range(NPH):
                    absacc_act(ext[:, 0:P, P * p:P * (p + 1)], exs[:, p:p + 1])

            # ey: eyt[q, c] = t[q+2, c] - t[q, c]  (row q+1), col c=j+1
            eyt = scr_pool.tile([128, P, WIN], sdt, tag="eyt")
            uidx = (s * 2 + h) * 2 + g
            on_pool = ey_eng[g] and uidx >= ey_s0_dve
            eng = nc.gpsimd if on_pool else nc.vector
            eng.tensor_tensor(
                flat(eyt[:, :, :], 0, K),
                flat(t3, 2 * WIN, K), flat(t3, 0, K),
                op=Alu.subtract,
            )
            if accey_eng[g]:
                absacc_dve(eyt[:, 0:P, 1:1 + NPH * P], NPH, WIN, eys[:, 0:NPH])
            else:
                for p in range(NPH):
                    absacc_act(eyt[:, 0:P, 1 + P * p:1 + P * (p + 1)],
                               eys[:, p:p + 1])

            # |x| patch sums
            if accx_eng[g]:
                absacc_dve(t[:, 1:1 + P, 1:1 + NPH * P], NPH, WIN, xs_[:, 0:NPH])
            else:
                for p in range(NPH):
                    absacc_act(t[:, 1:1 + P, 1 + P * p:1 + P * (p + 1)],
                               xs_[:, p:p + 1])
            return t, (exs, eys, xs_)

        def emit_sp(h, g, accs):
            exs, eys, xs_ = accs
            eng = nc.gpsimd if sp_pool else nc.vector
            spg = acc_pool.tile([128, NPH], f32, tag="sp", name=f"sp{g}{h}")
            if sp_pool:
                eng.tensor_tensor(spg[:], exs[:], eys[:], op=Alu.add)
            else:
                eng.scalar_tensor_tensor(
                    spg[:], exs[:], 1.0, eys[:], op0=Alu.mult, op1=Alu.add,
                )
            eng.tensor_tensor(spg[:], spg[:], xs_[:], op=Alu.add)
            return spg

        def emit_rank_store(s, h, xt, sp):
            r0 = s * P
            nsp = {}
            for g in range(2):
                if cnt_eng[g] == 0:
                    nspg = acc_pool.tile([128, NPH], f32, tag="nsp",
                                         name=f"nsp{g}{h}")
                    nc.vector.tensor_scalar(
                        nspg[:], sp[g][:], -1.0, None, op0=Alu.mult
                    )
                    nsp[g] = nspg

            spT = spt_pool.tile([NPH, 256], f32, tag="spT")
            for g in range(2):
                pt = ps_t.tile([NPH, 128], f32, tag="psT")
                nc.tensor.transpose(pt[:], sp[g][:], ident[:])
                nc.vector.tensor_copy(spT[:, g * 128:(g + 1) * 128], pt[:])

            sgn = [
                acc_pool.tile([128, NPH], f32, tag="sgn", name=f"sgn{g}{h}")
                for g in range(2)
            ]
            for p in range(NPH):
                pb = ps_b.tile([128, 256], f32, tag="pb")
                nc.tensor.matmul(
                    pb[:], onehot[:, 128 * p:128 * (p + 1)], spT[:],
                    start=True, stop=True,
                )
                for g in range(2):
                    if cnt_eng[g]:
                        nc.vector.tensor_scalar(
                            dve_dump2, pb[:], sp[g][:, p:p + 1], None,
                            op0=Alu.is_gt, op1=Alu.add,
                            accum_out=sgn[g][:, p:p + 1],
                        )
                    else:
                        po = ps_sgn.tile([128, 256], f32, tag="po")
                        nc.scalar.activation(
                            po[:], pb[:], Act.Sign,
                            bias=nsp[g][:, p:p + 1],
                            accum_out=sgn[g][:, p:p + 1],
                        )

            mask = []
            for g in range(2):
                mg = acc_pool.tile([128, NPH], f32, tag="mask",
                                   name=f"mask{g}{h}")
                thresh = 127.5 if cnt_eng[g] else -0.5
                nc.vector.tensor_scalar(
                    mg[:], sgn[g][:], thresh, None, op0=Alu.is_le
                )
                mask.append(mg)

            for g in range(2):
                t = xt[g]
                st = st_pool.tile([128, P, HW], f16, tag="st")
                for p in range(NPH):
                    reg = t[:, 1:1 + P, 1 + P * p:1 + P * (p + 1)]
                    dst = st[:, :, P * p:P * (p + 1)]
                    if mask_eng[g]:
                        nc.vector.tensor_scalar(
                            dst, reg, mask[g][:, p:p + 1], None, op0=Alu.mult,
                        )
                    else:
                        nc.scalar.activation(
                            dst, reg, Act.Copy, scale=mask[g][:, p:p + 1],
                        )

                def _issue(g=g, st=st, r0=r0, h=h):
                    part0 = h * C + g * 128
                    nc.scalar.dma_start(
                        o_ap[part0:part0 + 128, r0:r0 + P, :], st[:],
                    )
                if out_defer:
                    pending_outs.append(_issue)
                else:
                    _issue()

        pending_outs = []
        for s in range(ns):
            for h in range(2):
                xt = {}
                sp = {}
                accs = {}
                for g in range(2):
                    xt[g], accs[g] = emit_unit(s, h, g)
                for g in range(2):
                    sp[g] = emit_sp(h, g, accs[g])
                prev = list(pending_outs)
                pending_outs.clear()
                emit_rank_store(s, h, xt, sp)
                for f in prev:
                    f()
        for f in pending_outs:
            f()

    nc.compile()
    return nc


def onehot_input(nph=NPH):
    oh = np.zeros((nph, nph * 128), np.float32)
    for p in range(nph):
        oh[p, 128 * p:128 * (p + 1)] = 1.0
    return oh


BEST = dict(mask_eng=(1, 1), cnt_eng=(1, 0), sp_pool=0, ey_s0_dve=4,
            xbufs=4, scrbufs=3, stbufs=3)


def _get_program():
    key = "full"
    if key not in _cache:
        _cache[key] = build_program(**BEST)
    return _cache[key]


def make_in_maps(x):
    """Split x into 8 per-core maps: [2*256, 130, 130] f32 each."""
    xp = np.pad(x, ((0, 0), (0, 0), (1, 1), (1, 1)))
    oh = onehot_input()
    in_maps = []
    for n in range(N_VEH):
        for hh in range(2):
            slab = xp[n, :, hh * HS:hh * HS + HS + 2, :]   # [256,130,258]
            shard = np.empty((2 * C, HS + 2, WIN), np.float32)
            shard[:C] = slab[:, :, 0:WIN]
            shard[C:] = slab[:, :, HW:HW + WIN]
            in_maps.append({"x": shard, "onehot": oh})
    return in_maps


def kernel(x):
    """x: (4, 256, 256, 256) float32 -> masked output, same shape."""
    from concourse.bass_utils import run_bass_kernel_spmd

    x = np.asarray(x)
    assert x.shape == (N_VEH, C, H, W) and x.dtype == np.float32

    nc = _get_program()
    res = run_bass_kernel_spmd(nc, make_in_maps(x), list(range(N_CORES)))

    out = np.empty((N_VEH, C, H, W), np.float32)
    for n in range(N_VEH):
        for hh in range(2):
            o = res.results[n * 2 + hh]["out"]          # [512,128,128] f16
            o = o.astype(np.float32).reshape(2, C, HS, HW)
            out[n, :, hh * HS:(hh + 1) * HS, 0:HW] = o[0]
            out[n, :, hh * HS:(hh + 1) * HS, HW:W] = o[1]
    return out


# revision 4
# speedup vs baseline: 46903.1509x; 1.0133x over previous
"""Trainium2 Bass kernel v2 for ChannelSelection (top-k channel masking).

Reference computation (per vehicle n of N=4):
  s = 0.5*grad_mag(x) + 0.5*|x|            # grad_mag = |x[w+1]-x[w-1]| + |x[h+1]-x[h-1]|
  sp[c, patch] = mean of s over 32x32 patch
  keep top-128 (of 256) channels per patch (rank by sp desc)
  out = x * mask broadcast over patch

Differences vs v1 (378us): the host pre-splits each core's slab into two
width-halves with their own 1-px halo, stored contiguously, so every
input DMA is one ~17.7KB contiguous span per partition (one descriptor)
instead of 34 strided 520B rows; output is fp16 (rel err ~2e-4, within
the 2e-2 gate) and also lands as one 8KB span per partition. Subtracts
run as flat 1-D contiguous tensor_tensor over whole tiles (halo columns
produce garbage lanes that the patch-sum views never read).

Sharding: 8 cores = N(4) x H-halves(2). Per core: 16 units =
4 row-strips x 2 width-halves x 2 channel-groups, tile [128, 34, 130].

Engine split (knobs in BEST): DVE ex-subtract + |ey| patch-reduce +
maskmul; ACT |ex| + |x| patch-accumulate; Pool ey-subtract; PE rank
broadcast. Rank counts split DVE(is_gt)/ACT(Sign).
"""

import dataclasses
import sys

import numpy as np

_TRN_REPO = "/opt/trn_rl_repo"
if _TRN_REPO not in sys.path:
    sys.path.insert(0, _TRN_REPO)

N_VEH = 4
C = 256
H = 256
W = 256
P = 32
N_CORES = 8
HS = 128          # rows per core
NPH = 4           # patches per width-half
HW = 128          # out cols per half
WIN = 130         # in cols per half (1px halo each side)

_cache = {}


def build_program(xbufs=4, scrbufs=3, stbufs=3,
                  ex_eng=(0, 0), ey_eng=(1, 1),
                  accx_eng=(0, 0), accex_eng=(0, 0), accey_eng=(1, 1),
                  mask_eng=(1, 0), cnt_eng=(1, 0),
                  out_defer=1, ey_s0_dve=1, sp_pool=0, scr_f16=0):
    """One-core SPMD program.

    Engine codes, per channel-group g: subtracts 0=DVE 1=Pool;
    accs 0=ACT(per-patch activation Abs) 1=DVE(tensor_reduce XY abs);
    mask_eng 0=ACT(Copy scale) 1=DVE(tensor_scalar mult);
    cnt_eng 0=ACT(Sign bias accum) 1=DVE(is_gt accum).
    ey_s0_dve: strip-0 ey subs forced to DVE (Pool starts cold).
    """
    from contextlib import ExitStack

    import concourse.bass as bass
    import concourse.tile as tile
    from concourse import bacc, masks, mybir

    f32 = mybir.dt.float32
    f16 = mybir.dt.float16
    Alu = mybir.AluOpType
    Act = mybir.ActivationFunctionType

    ns = 4                      # strips
    rows = ns * P               # 128

    nc = bacc.Bacc("TRN2", target_bir_lowering=False, debug=False)
    # [half, ch] flattened: per-partition rows are contiguous spans
    x_ap = nc.dram_tensor("x", [2 * C, rows + 2, WIN], f32,
                          kind="ExternalInput").ap()
    oh_ap = nc.dram_tensor("onehot", [NPH, NPH * 128], f32,
                           kind="ExternalInput").ap()
    o_ap = nc.dram_tensor("out", [2 * C, rows, HW], f16,
                          kind="ExternalOutput").ap()

    with tile.TileContext(nc) as tc, ExitStack() as ctx:
        const_pool = ctx.enter_context(tc.tile_pool(name="const", bufs=1))
        x_pool = ctx.enter_context(tc.tile_pool(name="xs", bufs=xbufs))
        scr_pool = ctx.enter_context(tc.tile_pool(name="scr", bufs=scrbufs))
        st_pool = ctx.enter_context(tc.tile_pool(name="st", bufs=stbufs))
        acc_pool = ctx.enter_context(tc.tile_pool(name="acc", bufs=22))
        spt_pool = ctx.enter_context(tc.tile_pool(name="spt", bufs=2))
        dump_pool = ctx.enter_context(tc.tile_pool(name="dump", bufs=1))
        ps_b = ctx.enter_context(tc.tile_pool(name="ps_b", bufs=5, space="PSUM"))
        ps_sgn = ctx.enter_context(tc.tile_pool(name="ps_sgn", bufs=1, space="PSUM"))
        ps_t = ctx.enter_context(tc.tile_pool(name="ps_t", bufs=2, space="PSUM"))

        ident = const_pool.tile([128, 128], f32)
        masks.make_identity(nc, ident[:])
        onehot = const_pool.tile([NPH, NPH * 128], f32)
        nc.scalar.dma_start(onehot[:], oh_ap[:])

        dve_dump = dump_pool.tile([128, 256], f16, tag="dve_dump")
        act_dump = dump_pool.tile([128, P, P], f16, tag="act_dump")
        act_dump3 = act_dump[:, :, :]
        _dd = dve_dump[:, :]
        dve_dump2 = dataclasses.replace(_dd, ap=[_dd.ap[0], [1, 256]])

        def flat(ap3, offset, count):
            """Flat 1-D view of a [128, R, Ccols] tile AP from elem offset."""
            r0, c0 = divmod(offset, ap3.shape[2])
            sl = ap3[:, r0:, c0:]
            return dataclasses.replace(sl, ap=[sl.ap[0], [1, count]])

        def absacc_act(src3, accum):
            nc.scalar.activation(act_dump3, src3, Act.Abs, accum_out=accum)

        def absacc_dve(src, np_, row_stride, accum):
            v = dataclasses.replace(
                src, ap=[src.ap[0], [P, np_], [row_stride, P], [1, P]]
            )
            nc.vector.tensor_reduce(
                accum, v, axis=mybir.AxisListType.XY, op=Alu.add,
                apply_absolute_value=True,
            )

        def emit_unit(s, h, g):
            """Load + subtracts + patch-accumulates for one unit."""
            r0 = s * P
            part0 = h * C + g * 128
            t = x_pool.tile([128, P + 2, WIN], f32, tag="x", name=f"x{g}{h}")
            nc.sync.dma_start(t[:], x_ap[part0:part0 + 128, r0:r0 + P + 2, :])

            exs = acc_pool.tile([128, NPH], f32, tag="exs", name=f"exs{g}{h}")
            eys = acc_pool.tile([128, NPH], f32, tag="eys", name=f"eys{g}{h}")
            xs_ = acc_pool.tile([128, NPH], f32, tag="xs", name=f"xs{g}{h}")

            t3 = t[:, :, :]
            K = P * WIN  # 4160
            sdt = f16 if scr_f16 else f32

            # ex: ext[q, i] = t[q+1, i+2] - t[q+1, i]  (valid out-col i<=127)
            ext = scr_pool.tile([128, P, WIN], sdt, tag="ext")
            eng = nc.gpsimd if ex_eng[g] else nc.vector
            eng.tensor_tensor(
                flat(ext[:, :, :], 0, K),
                flat(t3, WIN + 2, K), flat(t3, WIN, K),
                op=Alu.subtract,
            )
            # |ex| patch sums
            if accex_eng[g]:
                absacc_dve(ext[:, 0:P, 0:NPH * P], NPH, WIN, exs[:, 0:NPH])
            else:
                for p in range(NPH):
                    absacc_act(ext[:, 0:P, P * p:P * (p + 1)], exs[:, p:p + 1])

            # ey: eyt[q, c] = t[q+2, c] - t[q, c]  (row q+1), col c=j+1
            eyt = scr_pool.tile([128, P, WIN], sdt, tag="eyt")
            uidx = (s * 2 + h) * 2 + g
            on_pool = ey_eng[g] and uidx >= ey_s0_dve
            eng = nc.gpsimd if on_pool else nc.vector
            eng.tensor_tensor(
                flat(eyt[:, :, :], 0, K),
                flat(t3, 2 * WIN, K), flat(t3, 0, K),
                op=Alu.subtract,
            )
            if accey_eng[g]:
                absacc_dve(eyt[:, 0:P, 1:1 + NPH * P], NPH, WIN, eys[:, 0:NPH])
            else:
                for p in range(NPH):
                    absacc_act(eyt[:, 0:P, 1 + P * p:1 + P * (p + 1)],
                               eys[:, p:p + 1])

            # |x| patch sums
            if accx_eng[g]:
                absacc_dve(t[:, 1:1 + P, 1:1 + NPH * P], NPH, WIN, xs_[:, 0:NPH])
            else:
                for p in range(NPH):
                    absacc_act(t[:, 1:1 + P, 1 + P * p:1 + P * (p + 1)],
                               xs_[:, p:p + 1])
            return t, (exs, eys, xs_)

        def emit_sp(h, g, accs):
            exs, eys, xs_ = accs
            eng = nc.gpsimd if sp_pool else nc.vector
            spg = acc_pool.tile([128, NPH], f32, tag="sp", name=f"sp{g}{h}")
            if sp_pool:
                eng.tensor_tensor(spg[:], exs[:], eys[:], op=Alu.add)
            else:
                eng.scalar_tensor_tensor(
                    spg[:], exs[:], 1.0, eys[:], op0=Alu.mult, op1=Alu.add,
                )
            eng.tensor_tensor(spg[:], spg[:], xs_[:], op=Alu.add)
            return spg

        def emit_rank_store(s, h, xt, sp):
            r0 = s * P
            nsp = {}
            for g in range(2):
                if cnt_eng[g] == 0:
                    nspg = acc_pool.tile([128, NPH], f32, tag="nsp",
                                         name=f"nsp{g}{h}")
                    nc.vector.tensor_scalar(
                        nspg[:], sp[g][:], -1.0, None, op0=Alu.mult
                    )
                    nsp[g] = nspg

            spT = spt_pool.tile([NPH, 256], f32, tag="spT")
            for g in range(2):
                pt = ps_t.tile([NPH, 128], f32, tag="psT")
                nc.tensor.transpose(pt[:], sp[g][:], ident[:])
                nc.vector.tensor_copy(spT[:, g * 128:(g + 1) * 128], pt[:])

            sgn = [
                acc_pool.tile([128, NPH], f32, tag="sgn", name=f"sgn{g}{h}")
                for g in range(2)
            ]
            for p in range(NPH):
                pb = ps_b.tile([128, 256], f32, tag="pb")
                nc.tensor.matmul(
                    pb[:], onehot[:, 128 * p:128 * (p + 1)], spT[:],
                    start=True, stop=True,
                )
                for g in range(2):
                    if cnt_eng[g]:
                        nc.vector.tensor_scalar(
                            dve_dump2, pb[:], sp[g][:, p:p + 1], None,
                            op0=Alu.is_gt, op1=Alu.add,
                            accum_out=sgn[g][:, p:p + 1],
                        )
                    else:
                        po = ps_sgn.tile([128, 256], f32, tag="po")
                        nc.scalar.activation(
                            po[:], pb[:], Act.Sign,
                            bias=nsp[g][:, p:p + 1],
                            accum_out=sgn[g][:, p:p + 1],
                        )

            mask = []
            for g in range(2):
                mg = acc_pool.tile([128, NPH], f32, tag="mask",
                                   name=f"mask{g}{h}")
                thresh = 127.5 if cnt_eng[g] else -0.5
                nc.vector.tensor_scalar(
                    mg[:], sgn[g][:], thresh, None, op0=Alu.is_le
                )
                mask.append(mg)

            for g in range(2):
                t = xt[g]
                st = st_pool.tile([128, P, HW], f16, tag="st")
                for p in range(NPH):
                    reg = t[:, 1:1 + P, 1 + P * p:1 + P * (p + 1)]
                    dst = st[:, :, P * p:P * (p + 1)]
                    if mask_eng[g]:
                        nc.vector.tensor_scalar(
                            dst, reg, mask[g][:, p:p + 1], None, op0=Alu.mult,
                        )
                    else:
                        nc.scalar.activation(
                            dst, reg, Act.Copy, scale=mask[g][:, p:p + 1],
                        )

                def _issue(g=g, st=st, r0=r0, h=h):
                    part0 = h * C + g * 128
                    nc.scalar.dma_start(
                        o_ap[part0:part0 + 128, r0:r0 + P, :], st[:],
                    )
                if out_defer:
                    pending_outs.append(_issue)
                else:
                    _issue()

        pending_outs = []
        for s in range(ns):
            for h in range(2):
                xt = {}
                sp = {}
                accs = {}
                for g in range(2):
                    xt[g], accs[g] = emit_unit(s, h, g)
                for g in range(2):
                    sp[g] = emit_sp(h, g, accs[g])
                prev = list(pending_outs)
                pending_outs.clear()
                emit_rank_store(s, h, xt, sp)
                for f in prev:
                    f()
        for f in pending_outs:
            f()

    nc.compile()
    return nc


def onehot_input(nph=NPH):
    oh = np.zeros((nph, nph * 128), np.float32)
    for p in range(nph):
        oh[p, 128 * p:128 * (p + 1)] = 1.0
    return oh


BEST = dict(mask_eng=(1, 1), cnt_eng=(1, 0), sp_pool=0, ey_s0_dve=4,
            accx_eng=(0, 1), xbufs=4, scrbufs=3, stbufs=3)


def _get_program():
    key = "full"
    if key not in _cache:
        _cache[key] = build_program(**BEST)
    return _cache[key]


def make_in_maps(x):
    """Split x into 8 per-core maps: [2*256, 130, 130] f32 each."""
    xp = np.pad(x, ((0, 0), (0, 0), (1, 1), (1, 1)))
    oh = onehot_input()
    in_maps = []
    for n in range(N_VEH):
        for hh in range(2):
            slab = xp[n, :, hh * HS:hh * HS + HS + 2, :]   # [256,130,258]
            shard = np.empty((2 * C, HS + 2, WIN), np.float32)
            shard[:C] = slab[:, :, 0:WIN]
            shard[C:] = slab[:, :, HW:HW + WIN]
            in_maps.append({"x": shard, "onehot": oh})
    return in_maps


def kernel(x):
    """x: (4, 256, 256, 256) float32 -> masked output, same shape."""
    from concourse.bass_utils import run_bass_kernel_spmd

    x = np.asarray(x)
    assert x.shape == (N_VEH, C, H, W) and x.dtype == np.float32

    nc = _get_program()
    res = run_bass_kernel_spmd(nc, make_in_maps(x), list(range(N_CORES)))

    out = np.empty((N_VEH, C, H, W), np.float32)
    for n in range(N_VEH):
        for hh in range(2):
            o = res.results[n * 2 + hh]["out"]          # [512,128,128] f16
            o = o.astype(np.float32).reshape(2, C, HS, HW)
            out[n, :, hh * HS:(hh + 1) * HS, 0:HW] = o[0]
            out[n, :, hh * HS:(hh + 1) * HS, HW:W] = o[1]
    return out
